# revision 34
# baseline (speedup 1.0000x reference)
"""Trainium2 Bass kernel for DifferentiableTopK (Sinkhorn top-k masking).

Math (per batch row s in R^n, n=2048, K=256, eps=1e-3): the reference builds
log_P[i,j] = -(s_i - sorted(s)_j)^2/eps, runs 2 Sinkhorn normalizations
(col then row), and returns logsumexp over the first K (sorted) columns.

Kernel strategy (per batch, sorted domain, x = sorted scores descending):
  G[a,b] = exp(-1000*(x_a-x_b)^2) is symmetric, so all Sinkhorn reductions
  are weighted row sums = TensorEngine matvecs against stored G tiles:
    S1 = G @ 1 ; w1 = 1/S1 ; S2 = G @ w1 ; w2 = 1/S2 ; S3 = G @ w2
    w3 = 1/S3 ; S4 = G @ w3
    M[a] = 0 if a<K else -1000*(x_a - x_{K-1})^2
    ET[b,a] = exp(-1000*(x_a-x_b)^2 - M[a]) for b<K ; Ksum = ET^T @ w3[:K]
    out_sorted[a] = M[a] + log(Ksum[a] / S4[a])

  G/ET are built on the TensorEngine as an outer-product expansion of the
  squared distance: t0 = x_a*(2000 x_b) + (-1000 x_b^2) (+ (-M[a]) for ET),
  with every factor split into 3 bf16 limbs so a single-pass bf16 matmul
  (K=9 for G, K=12 for ET) reproduces fp32-level accuracy; one ScalarEngine
  Exp (bias = -1000 x_a^2, the natural_log_exp_and_others table) finishes
  each tile in bf16. All work is band-limited at 256-column granularity:
  entries with |x_a - x_b| > 0.26 contribute < e^-67 to any sum and are
  skipped (the host unions coverage over all rows so one SPMD program
  serves all 8 cores). S1 falls out of the Exp's accum_out, reduced and
  reciprocated per quarter so each Sinkhorn pass starts before its build
  fully finishes. Matvecs keep G stationary (128x128 bf16 blocks) so
  results land partition-major in PSUM — no transposes anywhere. The
  batch loop is software-pipelined; the device ships q = Ksum/S4 and the
  host applies out = M + ln(q) (keeps the Ln table set off the device).

Sharding: pure data parallel, 32 rows -> 8 cores x 4. Host does the sort and
tiny per-row prep; device does all n^2 work; host inverse-permutes.
"""
import math
import sys

sys.path.insert(0, "/opt/trn_rl_repo")

import numpy as np
import ml_dtypes
from contextlib import ExitStack

import concourse.bass as bass
import concourse.mybir as mybir
from concourse import bacc, tile
from concourse.bass_utils import run_bass_kernel_spmd

N = 2048
B = 32
NCORES = 8
BPC = B // NCORES
K = 256
NBLK = N // 128   # 16 partition blocks
NCH = N // 512    # 4 build chunks
BAND = 0.23       # build band: entries beyond are < e^-52, invisible in the sums
MVBAND = 0.19     # matvec band (subset of BAND): dropped pairs ~100x below bf16 noise
ETLIM = 52.0      # ET entries with exponent < -52 are invisible in the sums
F32 = mybir.dt.float32
BF16 = mybir.dt.bfloat16
AF = mybir.ActivationFunctionType
BF = ml_dtypes.bfloat16


def _coverage(xs_all):
    """Union (over all 32 rows) band coverage per batch slot.

    cov512[b][m]: build chunks (of 4) needed for G block m.
    cov128[b][m]: contraction 128-blocks k for the S matvecs.
    etch[b][blk]: build chunks needed for ET block blk (b<K rows).
    etmv[b][m]:   ET blocks blk contributing to Ksum output block m.
    """
    def runs(chunks):
        """Sorted 256-col chunk ids -> (start, n) runs of <=4 chunks
        (a 4x256-col psum tile is 2 banks)."""
        out = []
        for c in sorted(chunks):
            if out and out[-1][0] + out[-1][1] == c and out[-1][1] < 4:
                out[-1] = (out[-1][0], out[-1][1] + 1)
            else:
                out.append((c, 1))
        return out

    cov512 = [[set() for _ in range(NBLK)] for _ in range(BPC)]
    cov128 = [[set() for _ in range(NBLK)] for _ in range(BPC)]
    etch = [[set() for _ in range(2)] for _ in range(BPC)]
    etmv = [[set() for _ in range(NBLK)] for _ in range(BPC)]
    for row in range(B):
        b = row % BPC
        x = xs_all[row].astype(np.float64)
        M = np.where(np.arange(N) < K, 0.0, 1000.0 * (x - x[K - 1]) ** 2)
        bhi = [x[m * 128] for m in range(NBLK)]
        blo = [x[m * 128 + 127] for m in range(NBLK)]
        for m in range(NBLK):
            for kb in range(NBLK):
                if not (blo[m] - bhi[kb] > MVBAND or blo[kb] - bhi[m] > MVBAND):
                    cov128[b][m].add(kb)
            for c in range(2 * NCH):
                chi, clo = x[c * 256], x[c * 256 + 255]
                if not (blo[m] - chi > BAND or clo - bhi[m] > BAND):
                    cov512[b][m].add(c)
        # ET: entry (bb, a) alive iff 1000*(x_a-x_bb)^2 + M[a] <= ETLIM
        for blk in range(2):
            xb = x[blk * 128:(blk + 1) * 128]
            lo_b, hi_b = xb[-1], xb[0]
            # min over bb in block of (x_a - x_bb)^2 = interval distance.
            # ET exponent is -1000*gap^2 + M (M = +1000*(x_a - tau)^2
            # compensates the distance for far a), so alive needs
            # 1000*gap^2 - M <= ETLIM.
            gap = np.maximum(np.maximum(lo_b - x, x - hi_b), 0.0)
            alive = 1000.0 * gap * gap - M <= ETLIM
            for c in range(2 * NCH):
                if alive[c * 256:(c + 1) * 256].any():
                    etch[b][blk].add(c)
            for m in range(NBLK):
                if alive[m * 128:(m + 1) * 128].any():
                    etmv[b][m].add(blk)
    def span(chunks):
        c = sorted(chunks)
        return (c[0], c[-1] - c[0] + 1)  # fill holes: one contiguous run
    srt = lambda ll: [[sorted(s) for s in row] for row in ll]
    sp = lambda ll: [[span(s) for s in row] for row in ll]
    rr = lambda ll: [[runs(s) for s in row] for row in ll]
    return sp(cov512), srt(cov128), rr(etch), srt(etmv)


def build_program(cov512, cov128, etch, etmv):
    nc = bacc.Bacc("TRN2", target_bir_lowering=False, debug=False)

    d_lhs = nc.dram_tensor("lhsb", [BPC, 12, N], BF16, kind="ExternalInput").ap()
    d_rhs = nc.dram_tensor("rhsb", [BPC, 12, N], BF16, kind="ExternalInput").ap()
    d_eb = nc.dram_tensor("ebias", [BPC, 128, NBLK], F32, kind="ExternalInput").ap()
    d_out = nc.dram_tensor("out", [BPC, 128, NBLK], F32, kind="ExternalOutput").ap()

    with tile.TileContext(nc) as tc:
        with ExitStack() as ctx:
            gp = ctx.enter_context(tc.tile_pool(name="gpool", bufs=2 * NBLK))
            etp = ctx.enter_context(tc.tile_pool(name="etpool", bufs=4))
            rows = ctx.enter_context(tc.tile_pool(name="rows", bufs=3))
            tiny = ctx.enter_context(tc.tile_pool(name="tiny", bufs=4))
            acc = ctx.enter_context(tc.tile_pool(name="acc", bufs=3))
            fin = ctx.enter_context(tc.tile_pool(name="fin", bufs=BPC))
            pb = ctx.enter_context(tc.tile_pool(name="pbuild", bufs=3, space="PSUM"))
            pv = ctx.enter_context(tc.tile_pool(name="pvec", bufs=2, space="PSUM"))

            lhs0 = rows.tile([12, N], BF16, tag="lhsb")
            nc.sync.dma_start(lhs0[:], d_lhs[0])
            rhs0 = rows.tile([12, N], BF16, tag="rhsb")
            nc.sync.dma_start(rhs0[:], d_rhs[0])

            state = {}

            def emit_build(b, lhs0, rhs0):
                if b == 0:
                    lhsb, rhsb = lhs0, rhs0
                else:
                    lhsb = rows.tile([12, N], BF16, tag="lhsb")
                    nc.sync.dma_start(lhsb[:], d_lhs[b])
                    rhsb = rows.tile([12, N], BF16, tag="rhsb")
                    nc.sync.dma_start(rhsb[:], d_rhs[b])
                eb = tiny.tile([128, NBLK], F32, tag="eb")
                nc.sync.dma_start(eb[:], d_eb[b])

                s1acc = acc.tile([128, NBLK * 2], F32, tag="s1acc")
                nc.gpsimd.memset(s1acc[:], 0.0)
                gt = []
                for m in range(NBLK):
                    g = gp.tile([128, N], BF16, tag="g")
                    c0, ln = cov512[b][m]
                    pieces = [(p, min(4, ln - p)) for p in range(0, ln, 4)]
                    for ri, (p0, pl) in enumerate(pieces):
                        ps = pb.tile([128, pl * 256], F32, tag="pb")
                        for j in range(pl):
                            nc.tensor.matmul(
                                ps[:, j * 256:(j + 1) * 256],
                                lhsb[0:9, m * 128:(m + 1) * 128],
                                rhsb[0:9, (c0 + p0 + j) * 256:
                                     (c0 + p0 + j + 1) * 256],
                                start=True, stop=True)
                        nc.scalar.activation(
                            g[:, (c0 + p0) * 256:(c0 + p0 + pl) * 256], ps[:],
                            AF.Exp, bias=eb[:, m:m + 1], scale=1.0,
                            accum_out=s1acc[:, m * 2 + ri:m * 2 + ri + 1])
                    gt.append(g)

                et = []
                for blk in range(2):
                    e = etp.tile([128, N], BF16, tag="et")
                    for (c0, ln) in etch[b][blk]:
                        ps = pb.tile([128, ln * 256], F32, tag="pb")
                        for j in range(ln):
                            nc.tensor.matmul(
                                ps[:, j * 256:(j + 1) * 256],
                                lhsb[0:12, blk * 128:(blk + 1) * 128],
                                rhsb[0:12, (c0 + j) * 256:(c0 + j + 1) * 256],
                                start=True, stop=True)
                        nc.scalar.activation(e[:, c0 * 256:(c0 + ln) * 256], ps[:],
                                             AF.Exp, bias=eb[:, blk:blk + 1],
                                             scale=1.0)
                    et.append(e)
                state[b] = (gt, et, s1acc)

            def emit_chain(b):
                gt, et, s1acc = state.pop(b)
                # reduce S1 per quarter so the S2 pass starts as soon as
                # the first blocks' builds (and their accums) are done
                s1h = []
                for h in range(4):
                    sh = tiny.tile([128, 4], F32, tag="s")
                    nc.vector.tensor_reduce(
                        sh[:], s1acc[:, h * 8:(h + 1) * 8].rearrange(
                            "p (m c) -> p m c", c=2),
                        axis=mybir.AxisListType.X, op=mybir.AluOpType.add)
                    s1h.append((sh[:], h * 4, 4))

                def recip_cast(parts):
                    wb = tiny.tile([128, NBLK], BF16, tag="wb")
                    for ps, c0w, wd in parts:
                        wf = tiny.tile([128, wd], F32, tag="wf")
                        nc.vector.reciprocal(wf[:], ps)
                        nc.vector.tensor_copy(wb[:, c0w:c0w + wd], wf[:])
                    return wb

                def matvec(wb):
                    halves = []
                    for h in range(2):
                        ps = pv.tile([128, 8], F32, tag="pv")
                        for mi in range(8):
                            m = h * 8 + mi
                            ks = cov128[b][m]
                            for i, kb in enumerate(ks):
                                nc.tensor.matmul(
                                    ps[:, mi:mi + 1],
                                    gt[kb][:, m * 128:(m + 1) * 128],
                                    wb[:, kb:kb + 1],
                                    start=(i == 0), stop=(i == len(ks) - 1))
                        halves.append(ps)
                    return halves

                w1 = recip_cast(s1h)
                ps2h = matvec(w1)
                w2 = recip_cast([(ps2h[0][:], 0, 8), (ps2h[1][:], 8, 8)])
                ps3h = matvec(w2)
                w3 = recip_cast([(ps3h[0][:], 0, 8), (ps3h[1][:], 8, 8)])
                ps4h = matvec(w3)

                q = fin.tile([128, NBLK], F32, tag="q")
                for h in range(2):
                    hs = slice(h * 8, (h + 1) * 8)
                    pk = pv.tile([128, 8], F32, tag="pv")
                    for mi in range(8):
                        m = h * 8 + mi
                        bs = etmv[b][m]
                        for i, blk in enumerate(bs):
                            nc.tensor.matmul(pk[:, mi:mi + 1],
                                             et[blk][:, m * 128:(m + 1) * 128],
                                             w3[:, blk:blk + 1],
                                             start=(i == 0), stop=(i == len(bs) - 1))
                    r4 = tiny.tile([128, 8], F32, tag="r4")
                    nc.vector.reciprocal(r4[:], ps4h[h][:])
                    nc.vector.tensor_mul(q[:, hs], pk[:], r4[:])
                nc.sync.dma_start(d_out[b], q[:])

            for b in range(BPC):
                emit_build(b, lhs0, rhs0)
                if b >= 1:
                    # chains preempt builds whenever their deps are ready;
                    # build matmuls fill the ACT-paced stalls.
                    with tc.high_priority():
                        emit_chain(b - 1)
            with tc.high_priority():
                emit_chain(BPC - 1)

    nc.compile()
    return nc


_CACHE = {}


def _limbs3(v):
    """Split fp32 array into 3 bf16 limbs (exact to ~2^-27 relative)."""
    v = v.astype(np.float32)
    l0 = v.astype(BF)
    r = v - l0.astype(np.float32)
    l1 = r.astype(BF)
    l2 = (r - l1.astype(np.float32)).astype(BF)
    return l0, l1, l2


def prepare(scores: np.ndarray):
    """Host prep: sort, coverage, program build, per-core input maps."""
    scores = np.ascontiguousarray(np.asarray(scores, dtype=np.float32))
    assert scores.shape == (B, N), scores.shape

    orders = np.argsort(-scores, axis=-1, kind="stable")
    xs = np.take_along_axis(scores, orders, axis=-1)  # [B, N] sorted desc

    covs = _coverage(xs)
    key = (xs.tobytes(),)
    if key not in _CACHE:
        _CACHE.clear()
        _CACHE[key] = build_program(*covs)
    nc = _CACHE[key]

    d_tau = xs - xs[:, K - 1:K]
    M = np.where(np.arange(N)[None, :] < K, np.float32(0.0),
                 (np.float32(-1000.0) * d_tau * d_tau).astype(np.float32)
                 ).astype(np.float32)

    a0, a1, a2 = _limbs3(xs)
    c0, c1, c2 = _limbs3(np.float32(2000.0) * xs)
    dd0, dd1, dd2 = _limbs3(np.float32(-1000.0) * xs * xs)
    m0, m1, m2 = _limbs3(-M)
    one = np.ones_like(xs).astype(BF)
    # K rows pair lhs[k] with rhs[k]; products a_i*c_j kept for i+j<=2.
    lhsb = np.stack([a0, a0, a0, a1, a1, a2, one, one, one, one, one, one],
                    axis=1)  # [B, 12, N] bf16
    rhsb = np.stack([c0, c1, c2, c0, c1, c0, dd0, dd1, dd2, m0, m1, m2],
                    axis=1)
    ebias = (np.float32(-1000.0) * xs * xs).astype(np.float32)

    def pm(a):
        return np.ascontiguousarray(a.reshape(B, NBLK, 128).transpose(0, 2, 1))

    eb_pm = pm(ebias)
    in_maps = []
    for c in range(NCORES):
        sl = slice(c * BPC, (c + 1) * BPC)
        in_maps.append({
            "lhsb": np.ascontiguousarray(lhsb[sl]),
            "rhsb": np.ascontiguousarray(rhsb[sl]),
            "ebias": np.ascontiguousarray(eb_pm[sl]),
        })
    return nc, in_maps, orders, M


def postprocess(results, orders, M):
    out = np.empty((B, N), dtype=np.float32)
    for c in range(NCORES):
        o = results[c]["out"]  # [BPC, 128, NBLK] = q, sorted-domain
        for b in range(BPC):
            gb = c * BPC + b
            q = np.ascontiguousarray(o[b].T).reshape(N).astype(np.float64)
            out[gb, orders[gb]] = (M[gb].astype(np.float64) + np.log(q)
                                   ).astype(np.float32)
    return out


def kernel(scores: np.ndarray) -> np.ndarray:
    nc, in_maps, orders, M = prepare(scores)
    res = run_bass_kernel_spmd(nc, in_maps, core_ids=list(range(NCORES)))
    return postprocess(res.results, orders, M)


if __name__ == "__main__":
    x = np.random.randn(B, N).astype(np.float32)
    y = kernel(x)
    print("kernel ran, out shape", y.shape, "finite:", np.isfinite(y).all())
